# revision 13
# baseline (speedup 1.0000x reference)
"""Trainium2 Bass kernel for nn_Blur: upfirdn2d 2x upsample with a separable
4-tap binomial FIR (depthwise), data-parallel over batch across 8 NeuronCores.

HBM traffic is the roofline (memory regime), so all HBM I/O is bf16 (the
2e-2 tolerance dwarfs bf16's ~2e-3): in 8.4 MB + out 33.6 MB per core
=> ~117 us at 358 GB/s, vs 226 us for the f32 version.

Math (per spatial dim, UP=2, KS=4, pad (1,2), taps h = [1,3,3,1]/8):
  out[2t]   = h1*x[t] + h3*x[t+1]        (h1=3/8, h3=1/8; x[H] = 0 pad)
  out[2t+1] = h0*x[t] + h2*x[t+1]        (h0=1/8, h2=3/8)

Per-core plan (2 images), groups of CG=8 channels with the image-row dim
H=128 on SBUF partitions; host supplies x as [n, h, c, w+pad] bf16:
  PE : Ve = (h1*B_e) @ x                       -> PSUM   (vertical, even rows)
       P1 = (h1*B_o) @ x + (h3*B_o) @ x>>1     -> PSUM   (odd rows, even cols)
       P2 = (h3*B_o) @ x + (h1*B_o) @ x>>1     -> PSUM   (odd rows, odd cols)
  ACT: tb = bf16(Ve) -> SBUF;  o[ro,ce] = bf16(P1)
  DVE: o[re,ce] = (tb>>1)*(1/3) + tb   (STT);  o[ro,co] = bf16(P2)
  GPS: o[re,co] = tb*(1/3) + (tb>>1)   (STT)
Output rows/cols are written phase-planar [c, h, r, q, w]; the host
interleaves columns, converts to f32 and crops to 255x255.
"""
import json

import numpy as np

import concourse.bass as bass
import concourse.mybir as mybir
from concourse.tile import TileContext

f32 = mybir.dt.float32
bf16 = mybir.dt.bfloat16

N, C, H, W = 16, 128, 128, 128
W1 = W + 1                   # input width padded with one zero column
OH, OW = 2 * H - 1, 2 * W - 1
NCORES = 8
NPER = N // NCORES           # images per core
CG = 8                       # channels per compute group
XCG = 64                     # channels per input DMA tile
OCG = 16                     # channels per output DMA


# ---------------------------------------------------------------------------
# The walrus in this container supports only ONE sync-wait command per
# instruction; Tile emits up to ~3. Post-process the serialized BIR: keep one
# wait per instruction, move the rest onto inserted same-engine NoOps.
def _split_waits(bir_json: bytes) -> bytes:
    d = json.loads(bir_json)
    ctr = 0
    for fn in d["functions"]:
        for blk in fn["blocks"]:
            out = []
            for inst in blk["instructions"]:
                si = inst.get("sync_info") or {}
                ow = si.get("on_wait") or []
                if len(ow) > 1:
                    for w in ow[:-1]:
                        ctr += 1
                        out.append({
                            "debug": inst.get("debug"),
                            "engine": inst["engine"],
                            "ins": [], "outs": [],
                            "name": f"WSPL-{ctr}",
                            "opcode": "NoOp",
                            "sync_info": {"on_update": [], "on_wait": [w]},
                        })
                    si["on_wait"] = ow[-1:]
                    inst["sync_info"] = si
                out.append(inst)
            blk["instructions"] = out
    return json.dumps(d).encode()


# ---------------------------------------------------------------------------
# Walrus in this container caps sync-wait commands per CTRL instruction; the
# stock TileContext end-of-kernel drain waits on every used proc lane at once
# and fails codegen. Split it into one drain per lane.
def _install_drain_patch():
    import concourse.tile as tile_mod
    from concourse.vector_clock import ScopedClock, VectorClock

    if getattr(tile_mod.TileContext, "_drain_split_patched", False):
        return

    def _split_drain(self, tick_clock, wait_clock):
        gc = tick_clock.global_clock
        ticks = list(gc)
        nz = [i for i, t in enumerate(ticks) if t > 0]
        for i in nz or [None]:
            vec = [0] * len(ticks)
            if i is not None:
                vec[i] = ticks[i]
            d = self.nc.sync.drain()
            wait_clock.add_sem_waits(d.ins, ScopedClock({None: VectorClock(vec)}))
        self.nc.all_engine_barrier()
        assert self.sems is not None
        popped = self.nc._tile_sem_poison_stack.pop()
        assert popped is self._sem_poison
        self.nc.clear_and_free_semaphores(list(self.sems.allocated().values()))
        self.nc.all_engine_barrier()

    tile_mod.TileContext._drain_and_barrier = _split_drain
    tile_mod.TileContext._drain_split_patched = True


def _build_program(reps: int = 1, nper: int = NPER, ch: int = C,
                   variant: str = "full", in_eng: str = "sync",
                   out_rings=("sync",), xbufs: int = 4, obufs: int = 3,
                   tbufs: int = 3, spbufs: int = 3, o_mode: str = "b"):
    """variant: 'full' | 'dma_only' (no compute) | 'no_out' | 'compute_only'.
    Non-full variants are for perf bisection only and produce garbage.

    o_mode 'a': GPS stages sp[j] = tb[j+1]/3 (aligned write); DVE does
      E = TT(tb, sp) [2x-eligible] and O = STT(tb/3 + tb>>1) [1x].
    o_mode 'b': GPS stages sb[j] = tb[j]/3; DVE does two TTs with 2-byte
      misaligned shifted reads (faster iff HW keeps 2x despite misalign)."""
    _install_drain_patch()
    nc = bass.Bass("TRN2")
    ngrp = ch // CG
    xcg = min(XCG, ch)
    ocg = min(OCG, ch)
    imgs = nc.dram_tensor("imgs", [nper, H, ch, W1], bf16, kind="ExternalInput")
    wmat = nc.dram_tensor("wmat", [3, H, H], bf16, kind="ExternalInput")
    # phase-planar output: [n, c, rowpair h, row-parity r, col-parity q, w];
    # host interleaves cols, crops row/col 255.
    out = nc.dram_tensor("out", [nper, ch, H, 2, 2, W], bf16,
                         kind="ExternalOutput")
    add = mybir.AluOpType.add
    mult = mybir.AluOpType.mult
    do_in = variant in ("full", "dma_only", "no_out")
    do_compute = variant in ("full", "no_out", "compute_only")
    do_out = variant in ("full", "dma_only")
    in_dma = getattr(nc, in_eng).dma_start

    with TileContext(nc) as tc:
        import contextlib
        rep_loop = tc.For_i(0, reps, 1) if reps > 1 else contextlib.nullcontext()
        with (
            tc.tile_pool(name="cpool", bufs=1) as cpool,
            tc.tile_pool(name="xp", bufs=xbufs) as xp,
            tc.tile_pool(name="tbp", bufs=tbufs) as tbp,
            tc.tile_pool(name="spp", bufs=spbufs) as spp,
            tc.tile_pool(name="pv", bufs=2, space="PSUM") as pv,
            tc.tile_pool(name="pp", bufs=1, space="PSUM") as pp,
            tc.tile_pool(name="op", bufs=obufs) as op,
            rep_loop,
        ):
            A = cpool.tile([128, 3, H], bf16)
            nc.sync.dma_start(A[:], wmat.rearrange("a k m -> k a m"))

            # hoist ALL input DMAs: they share one HWDGE FIFO ring with the
            # output DMAs, and queueing them mid-stream (behind pending
            # out-DMAs) drains the whole pipeline at each image boundary
            xtiles = {}
            for n in range(nper):
                for t in range(ngrp // (xcg // CG)):
                    x = xp.tile([128, xcg, W1], bf16, tag="x",
                                name=f"x_{n}_{t}")
                    xtiles[(n, t)] = x
                    if do_in:
                        in_dma(x[:], imgs[n, :, t * xcg:(t + 1) * xcg, :])
                    elif do_compute:
                        nc.vector.memset(x[:, :, 0:1], 0.0)
            n_odma = 0
            for n in range(nper):
                for g in range(ngrp):
                    c0 = CG * g
                    x = xtiles[(n, g // (xcg // CG))]
                    cs = c0 % xcg
                    xs0 = x[:, cs:cs + CG, 0:W]
                    xs1 = x[:, cs:cs + CG, 1:W1]
                    if g % (ocg // CG) == 0:
                        o = op.tile([128, ocg, 2, 2, W], bf16, tag="o")
                        if do_out and not do_compute:
                            nc.vector.memset(o[:, :, :, :, 0:1], 0.0)
                    co = c0 % ocg
                    os_ = o[:, co:co + CG]
                    if do_compute:
                        Ve = pv.tile([128, CG, W], f32, tag="Ve")
                        P1 = pp.tile([128, CG, W], f32, tag="P1")
                        P2 = pp.tile([128, CG, W], f32, tag="P2")
                        # one matmul output must stay within one 2KB PSUM
                        # bank => split each into [*, 4ch, W] (512 f32) halves
                        for hh in range(0, CG, 4):
                            s4 = slice(hh, hh + 4)
                            m0 = x[:, cs + hh:cs + hh + 4, 0:W]
                            m1 = x[:, cs + hh:cs + hh + 4, 1:W1]
                            nc.tensor.matmul(Ve[:, s4], A[:, 0], m0, start=True, stop=True)
                            nc.tensor.matmul(P1[:, s4], A[:, 1], m0, start=True, stop=False)
                            nc.tensor.matmul(P1[:, s4], A[:, 2], m1, start=False, stop=True)
                            nc.tensor.matmul(P2[:, s4], A[:, 2], m0, start=True, stop=False)
                            nc.tensor.matmul(P2[:, s4], A[:, 1], m1, start=False, stop=True)
                        tb = tbp.tile([128, CG, W1], bf16, tag="tb")
                        nc.vector.memset(tb[:, :, W:W1], 0.0)
                        nc.scalar.copy(tb[:, :, 0:W], Ve[:])
                        sp = spp.tile([128, CG, W1], bf16, tag="sp")
                        if o_mode == "b":
                            nc.vector.memset(sp[:, :, W:W1], 0.0)
                        # even rows: e-cols t[j] + t[j+1]/3, o-cols t[j]/3 + t[j+1]
                        if o_mode == "a":
                            nc.gpsimd.tensor_scalar(
                                out=sp[:, :, 0:W], in0=tb[:, :, 1:W1],
                                scalar1=1.0 / 3.0, scalar2=None, op0=mult)
                            nc.vector.scalar_tensor_tensor(
                                os_[:, :, 0, 1, :], tb[:, :, 0:W], 1.0 / 3.0,
                                tb[:, :, 1:W1], mult, add)
                            nc.vector.tensor_tensor(
                                os_[:, :, 0, 0, :], tb[:, :, 0:W],
                                sp[:, :, 0:W], add)
                        else:  # 'b': shifted reads keep 2x despite misalign
                            nc.gpsimd.tensor_scalar(
                                out=sp[:, :, 0:W], in0=tb[:, :, 0:W],
                                scalar1=1.0 / 3.0, scalar2=None, op0=mult)
                            nc.vector.tensor_tensor(
                                os_[:, :, 0, 0, :], tb[:, :, 0:W],
                                sp[:, :, 1:W1], add)
                            nc.vector.tensor_tensor(
                                os_[:, :, 0, 1, :], sp[:, :, 0:W],
                                tb[:, :, 1:W1], add)
                        # odd rows: evacuate the PE phase outputs
                        nc.scalar.copy(os_[:, :, 1, 0, :], P1[:])
                        nc.vector.tensor_copy(os_[:, :, 1, 1, :], P2[:])
                    if do_out and (g + 1) % (ocg // CG) == 0:
                        oc0 = c0 + CG - ocg
                        eng = getattr(nc, out_rings[n_odma % len(out_rings)])
                        n_odma += 1
                        eng.dma_start(
                            out[n, oc0:oc0 + ocg].rearrange(
                                "c h r q w -> h c (r q w)"),
                            o.rearrange("p c r q w -> p c (r q w)"),
                        )

    _orig = nc.to_json_bytes
    nc.to_json_bytes = lambda: _split_waits(bytes(_orig()))
    return nc


def _make_wmat(kernel4x4: np.ndarray) -> np.ndarray:
    """Stationary matrices [3, K=H, M=H]: banded vertical polyphase filters,
    pre-scaled by the horizontal taps so the width pass is t + t'/3 (even
    cols) and t/3 + t' (odd cols), with the 1/3 applied by the STT ops.
      A0 = h1*B_e (for t),  A1 = h1*B_o,  A2 = h3*B_o  (odd-row matmuls)
    All entries are k/64, exact in bf16."""
    k4 = np.asarray(kernel4x4, dtype=np.float64)
    k1 = k4[0, :] / np.sqrt(k4[0, 0])  # separable factor [1,3,3,1]/8
    h0, h1, h2, h3 = k1
    A = np.zeros((3, H, H), dtype=np.float64)
    idx = np.arange(H)
    i1 = idx[:-1]
    A[0, idx, idx] = h1 * h1
    A[0, i1 + 1, i1] = h1 * h3
    A[1, idx, idx] = h1 * h0
    A[1, i1 + 1, i1] = h1 * h2
    A[2, idx, idx] = h3 * h0
    A[2, i1 + 1, i1] = h3 * h2
    return A


_CACHE = {}


def _get_exec():
    """Compile the bass program and wrap it in a cached sharded jit callable
    (mirrors bass2jax.run_bass_via_pjrt's multi-core path, minus donation so
    the callable is reusable)."""
    if "fn" in _CACHE:
        return _CACHE["fn"]
    import jax
    from jax.sharding import Mesh, PartitionSpec, NamedSharding
    from jax.experimental.shard_map import shard_map
    from concourse import bass2jax

    nc = _build_program()
    bass2jax.install_neuronx_cc_hook()
    partition_name = nc.partition_id_tensor.name if nc.partition_id_tensor else None

    in_names, out_names, out_avals = [], [], []
    for alloc in nc.m.functions[0].allocations:
        if not isinstance(alloc, mybir.MemoryLocationSet):
            continue
        name = alloc.memorylocations[0].name
        if alloc.kind == "ExternalInput":
            if name != partition_name:
                in_names.append(name)
        elif alloc.kind == "ExternalOutput":
            out_names.append(name)
            out_avals.append(jax.core.ShapedArray(
                tuple(alloc.tensor_shape), mybir.dt.np(alloc.dtype)))
    all_in_names = list(in_names) + list(out_names)
    if partition_name is not None:
        all_in_names.append(partition_name)
    n_params = len(in_names)
    n_outs = len(out_avals)

    def _body(*args):
        operands = list(args)
        if partition_name is not None:
            operands.append(bass2jax.partition_id_tensor())
        return tuple(bass2jax._bass_exec_p.bind(
            *operands,
            out_avals=tuple(out_avals),
            in_names=tuple(all_in_names),
            out_names=tuple(out_names),
            lowering_input_output_aliases=(),
            sim_require_finite=True,
            sim_require_nnan=True,
            nc=nc,
        ))

    devices = jax.devices()[:NCORES]
    mesh = Mesh(np.asarray(devices), ("core",))
    fn = jax.jit(
        shard_map(_body, mesh=mesh,
                  in_specs=(PartitionSpec("core"),) * (n_params + n_outs),
                  out_specs=(PartitionSpec("core"),) * n_outs,
                  check_rep=False),
        keep_unused=True,
    )
    sharding = NamedSharding(mesh, PartitionSpec("core"))
    zeros = [np.zeros((NCORES * a.shape[0], *a.shape[1:]), a.dtype) for a in out_avals]
    _CACHE["fn"] = (fn, in_names, sharding, zeros)
    return _CACHE["fn"]


def _prep_imgs(imgs: np.ndarray) -> np.ndarray:
    """f32 [N, C, H, W] -> bf16 [N, H, C, W+1] with a zero pad column."""
    import ml_dtypes
    xin = np.zeros((N, H, C, W1), dtype=ml_dtypes.bfloat16)
    xin[..., :W] = imgs.transpose(0, 2, 1, 3)
    return xin


def _post_out(buf: np.ndarray) -> np.ndarray:
    """bf16 [N, C, H, 2, 2, W] phase-planar -> f32 [N, C, 255, 255]."""
    arr = buf.reshape(N, C, H, 2, 2, W).transpose(0, 1, 2, 3, 5, 4)
    full = arr.reshape(N, C, 2 * H, 2 * W)
    return np.ascontiguousarray(full[:, :, :OH, :OW]).astype(np.float32)


def kernel(**inputs) -> np.ndarray:
    import jax
    import ml_dtypes
    imgs = np.ascontiguousarray(np.asarray(inputs["imgs"], dtype=np.float32))
    kern = np.asarray(inputs["kernel"], dtype=np.float32)
    assert imgs.shape == (N, C, H, W), imgs.shape

    fn, in_names, sharding, zeros = _get_exec()
    wmat = _make_wmat(kern).astype(ml_dtypes.bfloat16)
    by_name = {
        "imgs": _prep_imgs(imgs),  # leading batch axis: shard_map splits it
        "wmat": np.concatenate([wmat] * NCORES, axis=0),
    }
    args = [jax.device_put(by_name[nm], sharding) for nm in in_names]
    zargs = [jax.device_put(z, sharding) for z in zeros]
    outs = fn(*args, *zargs)
    return _post_out(np.asarray(outs[0]))


# revision 15
# speedup vs baseline: 3.9552x; 3.9552x over previous
"""Trainium2 Bass kernel for nn_Blur: upfirdn2d 2x upsample with a separable
4-tap binomial FIR (depthwise), data-parallel over batch across 8 NeuronCores.

HBM traffic is the roofline (memory regime), so all HBM I/O is bf16 (the
2e-2 tolerance dwarfs bf16's ~2e-3): in 8.4 MB + out 33.6 MB per core
=> ~117 us at 358 GB/s, vs 226 us for the f32 version.

Math (per spatial dim, UP=2, KS=4, pad (1,2), taps h = [1,3,3,1]/8):
  out[2t]   = h1*x[t] + h3*x[t+1]        (h1=3/8, h3=1/8; x[H] = 0 pad)
  out[2t+1] = h0*x[t] + h2*x[t+1]        (h0=1/8, h2=3/8)

Per-core plan (2 images), groups of CG=8 channels with the image-row dim
H=128 on SBUF partitions; host supplies x as [n, h, c, w+pad] bf16:
  PE : Ve = (h1*B_e) @ x                       -> PSUM   (vertical, even rows)
       P1 = (h1*B_o) @ x + (h3*B_o) @ x>>1     -> PSUM   (odd rows, even cols)
       P2 = (h3*B_o) @ x + (h1*B_o) @ x>>1     -> PSUM   (odd rows, odd cols)
  ACT: tb = bf16(Ve) -> SBUF;  o[ro,ce] = bf16(P1)
  DVE: o[re,ce] = (tb>>1)*(1/3) + tb   (STT);  o[ro,co] = bf16(P2)
  GPS: o[re,co] = tb*(1/3) + (tb>>1)   (STT)
Output rows/cols are written phase-planar [c, h, r, q, w]; the host
interleaves columns, converts to f32 and crops to 255x255.
"""
import json

import numpy as np

import concourse.bass as bass
import concourse.mybir as mybir
from concourse.tile import TileContext

f32 = mybir.dt.float32
bf16 = mybir.dt.bfloat16

N, C, H, W = 16, 128, 128, 128
W1 = W + 1                   # input width padded with one zero column
OH, OW = 2 * H - 1, 2 * W - 1
NCORES = 8
NPER = N // NCORES           # images per core
CG = 8                       # channels per compute group
XCG = 64                     # channels per input DMA tile
OCG = 16                     # channels per output DMA


# ---------------------------------------------------------------------------
# The walrus in this container supports only ONE sync-wait command per
# instruction; Tile emits up to ~3. Post-process the serialized BIR: keep one
# wait per instruction, move the rest onto inserted same-engine NoOps.
def _split_waits(bir_json: bytes) -> bytes:
    d = json.loads(bir_json)
    ctr = 0
    for fn in d["functions"]:
        for blk in fn["blocks"]:
            out = []
            for inst in blk["instructions"]:
                si = inst.get("sync_info") or {}
                ow = si.get("on_wait") or []
                if len(ow) > 1:
                    for w in ow[:-1]:
                        ctr += 1
                        out.append({
                            "debug": inst.get("debug"),
                            "engine": inst["engine"],
                            "ins": [], "outs": [],
                            "name": f"WSPL-{ctr}",
                            "opcode": "NoOp",
                            "sync_info": {"on_update": [], "on_wait": [w]},
                        })
                    si["on_wait"] = ow[-1:]
                    inst["sync_info"] = si
                out.append(inst)
            blk["instructions"] = out
    return json.dumps(d).encode()


# ---------------------------------------------------------------------------
# Walrus in this container caps sync-wait commands per CTRL instruction; the
# stock TileContext end-of-kernel drain waits on every used proc lane at once
# and fails codegen. Split it into one drain per lane.
def _install_drain_patch():
    import concourse.tile as tile_mod
    from concourse.vector_clock import ScopedClock, VectorClock

    if getattr(tile_mod.TileContext, "_drain_split_patched", False):
        return

    def _split_drain(self, tick_clock, wait_clock):
        gc = tick_clock.global_clock
        ticks = list(gc)
        nz = [i for i, t in enumerate(ticks) if t > 0]
        for i in nz or [None]:
            vec = [0] * len(ticks)
            if i is not None:
                vec[i] = ticks[i]
            d = self.nc.sync.drain()
            wait_clock.add_sem_waits(d.ins, ScopedClock({None: VectorClock(vec)}))
        self.nc.all_engine_barrier()
        assert self.sems is not None
        popped = self.nc._tile_sem_poison_stack.pop()
        assert popped is self._sem_poison
        self.nc.clear_and_free_semaphores(list(self.sems.allocated().values()))
        self.nc.all_engine_barrier()

    tile_mod.TileContext._drain_and_barrier = _split_drain
    tile_mod.TileContext._drain_split_patched = True


def _build_program(reps: int = 1, nper: int = NPER, ch: int = C,
                   variant: str = "full", in_eng: str = "sync",
                   out_rings=("sync",), xbufs: int = 4, obufs: int = 3,
                   tbufs: int = 3, spbufs: int = 3, o_mode: str = "c"):
    """variant: 'full' | 'dma_only' (no compute) | 'no_out' | 'compute_only'.
    Non-full variants are for perf bisection only and produce garbage.

    o_mode 'a': GPS stages sp[j] = tb[j+1]/3 (aligned write); DVE does
      E = TT(tb, sp) [2x-eligible] and O = STT(tb/3 + tb>>1) [1x].
    o_mode 'b': GPS stages sb[j] = tb[j]/3; DVE does two TTs with 2-byte
      misaligned shifted reads (faster iff HW keeps 2x despite misalign)."""
    _install_drain_patch()
    nc = bass.Bass("TRN2")
    ngrp = ch // CG
    xcg = min(XCG, ch)
    ocg = min(OCG, ch)
    imgs = nc.dram_tensor("imgs", [nper, H, ch, W1], bf16, kind="ExternalInput")
    wmat = nc.dram_tensor("wmat", [3, H, H], bf16, kind="ExternalInput")
    # phase-planar output: [n, c, rowpair h, row-parity r, col-parity q, w];
    # host interleaves cols, crops row/col 255.
    out = nc.dram_tensor("out", [nper, ch, H, 2, 2, W], bf16,
                         kind="ExternalOutput")
    add = mybir.AluOpType.add
    mult = mybir.AluOpType.mult
    do_in = variant in ("full", "dma_only", "no_out")
    do_compute = variant in ("full", "no_out", "compute_only")
    do_out = variant in ("full", "dma_only")
    in_dma = getattr(nc, in_eng).dma_start

    with TileContext(nc) as tc:
        import contextlib
        rep_loop = tc.For_i(0, reps, 1) if reps > 1 else contextlib.nullcontext()
        with (
            tc.tile_pool(name="cpool", bufs=1) as cpool,
            tc.tile_pool(name="xp", bufs=xbufs) as xp,
            tc.tile_pool(name="tbp", bufs=tbufs) as tbp,
            tc.tile_pool(name="spp", bufs=spbufs) as spp,
            tc.tile_pool(name="pv", bufs=2, space="PSUM") as pv,
            tc.tile_pool(name="pp", bufs=1, space="PSUM") as pp,
            tc.tile_pool(name="op", bufs=obufs) as op,
            rep_loop,
        ):
            A = cpool.tile([128, 3, H], bf16)
            nc.sync.dma_start(A[:], wmat.rearrange("a k m -> k a m"))

            # hoist ALL input DMAs: they share one HWDGE FIFO ring with the
            # output DMAs, and queueing them mid-stream (behind pending
            # out-DMAs) drains the whole pipeline at each image boundary
            xtiles = {}
            for n in range(nper):
                for t in range(ngrp // (xcg // CG)):
                    x = xp.tile([128, xcg, W1], bf16, tag="x",
                                name=f"x_{n}_{t}")
                    xtiles[(n, t)] = x
                    if do_in:
                        in_dma(x[:], imgs[n, :, t * xcg:(t + 1) * xcg, :])
                    elif do_compute:
                        nc.vector.memset(x[:, :, 0:1], 0.0)
            n_odma = 0
            for n in range(nper):
                for g in range(ngrp):
                    c0 = CG * g
                    x = xtiles[(n, g // (xcg // CG))]
                    cs = c0 % xcg
                    xs0 = x[:, cs:cs + CG, 0:W]
                    xs1 = x[:, cs:cs + CG, 1:W1]
                    if g % (ocg // CG) == 0:
                        o = op.tile([128, ocg, 2, 2, W], bf16, tag="o")
                        if do_out and not do_compute:
                            nc.vector.memset(o[:, :, :, :, 0:1], 0.0)
                    co = c0 % ocg
                    os_ = o[:, co:co + CG]
                    if do_compute:
                        Ve = pv.tile([128, CG, W], f32, tag="Ve")
                        P1 = pp.tile([128, CG, W], f32, tag="P1")
                        P2 = pp.tile([128, CG, W], f32, tag="P2")
                        # one matmul output must stay within one 2KB PSUM
                        # bank => split each into [*, 4ch, W] (512 f32) halves
                        for hh in range(0, CG, 4):
                            s4 = slice(hh, hh + 4)
                            m0 = x[:, cs + hh:cs + hh + 4, 0:W]
                            m1 = x[:, cs + hh:cs + hh + 4, 1:W1]
                            nc.tensor.matmul(Ve[:, s4], A[:, 0], m0, start=True, stop=True)
                            nc.tensor.matmul(P1[:, s4], A[:, 1], m0, start=True, stop=False)
                            nc.tensor.matmul(P1[:, s4], A[:, 2], m1, start=False, stop=True)
                            nc.tensor.matmul(P2[:, s4], A[:, 2], m0, start=True, stop=False)
                            nc.tensor.matmul(P2[:, s4], A[:, 1], m1, start=False, stop=True)
                        tb = tbp.tile([128, CG, W1], bf16, tag="tb")
                        nc.vector.memset(tb[:, :, W:W1], 0.0)
                        nc.scalar.copy(tb[:, :, 0:W], Ve[:])
                        sp = spp.tile([128, CG, W1], bf16, tag="sp")
                        if o_mode in ("b", "c"):
                            nc.vector.memset(sp[:, :, W:W1], 0.0)
                        # even rows: e-cols t[j] + t[j+1]/3, o-cols t[j]/3 + t[j+1]
                        if o_mode == "c":
                            # GPS-free: DVE stages sb itself (tensor_scalar
                            # runs 4x on bf16 SBUF), then two 2x TTs
                            nc.vector.tensor_scalar(
                                out=sp[:, :, 0:W], in0=tb[:, :, 0:W],
                                scalar1=1.0 / 3.0, scalar2=None, op0=mult)
                            nc.vector.tensor_tensor(
                                os_[:, :, 0, 0, :], tb[:, :, 0:W],
                                sp[:, :, 1:W1], add)
                            nc.vector.tensor_tensor(
                                os_[:, :, 0, 1, :], sp[:, :, 0:W],
                                tb[:, :, 1:W1], add)
                        elif o_mode == "a":
                            nc.gpsimd.tensor_scalar(
                                out=sp[:, :, 0:W], in0=tb[:, :, 1:W1],
                                scalar1=1.0 / 3.0, scalar2=None, op0=mult)
                            nc.vector.scalar_tensor_tensor(
                                os_[:, :, 0, 1, :], tb[:, :, 0:W], 1.0 / 3.0,
                                tb[:, :, 1:W1], mult, add)
                            nc.vector.tensor_tensor(
                                os_[:, :, 0, 0, :], tb[:, :, 0:W],
                                sp[:, :, 0:W], add)
                        else:  # 'b': shifted reads keep 2x despite misalign
                            nc.gpsimd.tensor_scalar(
                                out=sp[:, :, 0:W], in0=tb[:, :, 0:W],
                                scalar1=1.0 / 3.0, scalar2=None, op0=mult)
                            nc.vector.tensor_tensor(
                                os_[:, :, 0, 0, :], tb[:, :, 0:W],
                                sp[:, :, 1:W1], add)
                            nc.vector.tensor_tensor(
                                os_[:, :, 0, 1, :], sp[:, :, 0:W],
                                tb[:, :, 1:W1], add)
                        # odd rows: evacuate the PE phase outputs
                        nc.scalar.copy(os_[:, :, 1, 0, :], P1[:])
                        nc.vector.tensor_copy(os_[:, :, 1, 1, :], P2[:])
                    if do_out and (g + 1) % (ocg // CG) == 0:
                        oc0 = c0 + CG - ocg
                        eng = getattr(nc, out_rings[n_odma % len(out_rings)])
                        n_odma += 1
                        eng.dma_start(
                            out[n, oc0:oc0 + ocg].rearrange(
                                "c h r q w -> h c (r q w)"),
                            o.rearrange("p c r q w -> p c (r q w)"),
                        )

    _orig = nc.to_json_bytes
    nc.to_json_bytes = lambda: _split_waits(bytes(_orig()))
    return nc


def _make_wmat(kernel4x4: np.ndarray) -> np.ndarray:
    """Stationary matrices [3, K=H, M=H]: banded vertical polyphase filters,
    pre-scaled by the horizontal taps so the width pass is t + t'/3 (even
    cols) and t/3 + t' (odd cols), with the 1/3 applied by the STT ops.
      A0 = h1*B_e (for t),  A1 = h1*B_o,  A2 = h3*B_o  (odd-row matmuls)
    All entries are k/64, exact in bf16."""
    k4 = np.asarray(kernel4x4, dtype=np.float64)
    k1 = k4[0, :] / np.sqrt(k4[0, 0])  # separable factor [1,3,3,1]/8
    h0, h1, h2, h3 = k1
    A = np.zeros((3, H, H), dtype=np.float64)
    idx = np.arange(H)
    i1 = idx[:-1]
    A[0, idx, idx] = h1 * h1
    A[0, i1 + 1, i1] = h1 * h3
    A[1, idx, idx] = h1 * h0
    A[1, i1 + 1, i1] = h1 * h2
    A[2, idx, idx] = h3 * h0
    A[2, i1 + 1, i1] = h3 * h2
    return A


_CACHE = {}


def _get_exec():
    """Compile the bass program and wrap it in a cached sharded jit callable
    (mirrors bass2jax.run_bass_via_pjrt's multi-core path, minus donation so
    the callable is reusable)."""
    if "fn" in _CACHE:
        return _CACHE["fn"]
    import jax
    from jax.sharding import Mesh, PartitionSpec, NamedSharding
    from jax.experimental.shard_map import shard_map
    from concourse import bass2jax

    nc = _build_program()
    bass2jax.install_neuronx_cc_hook()
    partition_name = nc.partition_id_tensor.name if nc.partition_id_tensor else None

    in_names, out_names, out_avals = [], [], []
    for alloc in nc.m.functions[0].allocations:
        if not isinstance(alloc, mybir.MemoryLocationSet):
            continue
        name = alloc.memorylocations[0].name
        if alloc.kind == "ExternalInput":
            if name != partition_name:
                in_names.append(name)
        elif alloc.kind == "ExternalOutput":
            out_names.append(name)
            out_avals.append(jax.core.ShapedArray(
                tuple(alloc.tensor_shape), mybir.dt.np(alloc.dtype)))
    all_in_names = list(in_names) + list(out_names)
    if partition_name is not None:
        all_in_names.append(partition_name)
    n_params = len(in_names)
    n_outs = len(out_avals)

    def _body(*args):
        operands = list(args)
        if partition_name is not None:
            operands.append(bass2jax.partition_id_tensor())
        return tuple(bass2jax._bass_exec_p.bind(
            *operands,
            out_avals=tuple(out_avals),
            in_names=tuple(all_in_names),
            out_names=tuple(out_names),
            lowering_input_output_aliases=(),
            sim_require_finite=True,
            sim_require_nnan=True,
            nc=nc,
        ))

    devices = jax.devices()[:NCORES]
    mesh = Mesh(np.asarray(devices), ("core",))
    fn = jax.jit(
        shard_map(_body, mesh=mesh,
                  in_specs=(PartitionSpec("core"),) * (n_params + n_outs),
                  out_specs=(PartitionSpec("core"),) * n_outs,
                  check_rep=False),
        keep_unused=True,
    )
    sharding = NamedSharding(mesh, PartitionSpec("core"))
    zeros = [np.zeros((NCORES * a.shape[0], *a.shape[1:]), a.dtype) for a in out_avals]
    _CACHE["fn"] = (fn, in_names, sharding, zeros)
    return _CACHE["fn"]


def _prep_imgs(imgs: np.ndarray) -> np.ndarray:
    """f32 [N, C, H, W] -> bf16 [N, H, C, W+1] with a zero pad column."""
    import ml_dtypes
    xin = np.zeros((N, H, C, W1), dtype=ml_dtypes.bfloat16)
    xin[..., :W] = imgs.transpose(0, 2, 1, 3)
    return xin


def _post_out(buf: np.ndarray) -> np.ndarray:
    """bf16 [N, C, H, 2, 2, W] phase-planar -> f32 [N, C, 255, 255]."""
    arr = buf.reshape(N, C, H, 2, 2, W).transpose(0, 1, 2, 3, 5, 4)
    full = arr.reshape(N, C, 2 * H, 2 * W)
    return np.ascontiguousarray(full[:, :, :OH, :OW]).astype(np.float32)


def kernel(**inputs) -> np.ndarray:
    import jax
    import ml_dtypes
    imgs = np.ascontiguousarray(np.asarray(inputs["imgs"], dtype=np.float32))
    kern = np.asarray(inputs["kernel"], dtype=np.float32)
    assert imgs.shape == (N, C, H, W), imgs.shape

    fn, in_names, sharding, zeros = _get_exec()
    wmat = _make_wmat(kern).astype(ml_dtypes.bfloat16)
    by_name = {
        "imgs": _prep_imgs(imgs),  # leading batch axis: shard_map splits it
        "wmat": np.concatenate([wmat] * NCORES, axis=0),
    }
    args = [jax.device_put(by_name[nm], sharding) for nm in in_names]
    zargs = [jax.device_put(z, sharding) for z in zeros]
    outs = fn(*args, *zargs)
    return _post_out(np.asarray(outs[0]))
